# revision 2
# baseline (speedup 1.0000x reference)
"""Trainium2 Bass kernel for nn_MultiHeadAttention (B=2, S=2048, D=2048, H=16, Dh=128).

Sharding: tensor-parallel over heads — 2 heads per core on 8 cores.
Each core computes q/k/v projections for its 2 heads (full 2048-deep
contraction in fp32r), RoPE, causal attention (bf16 internals, fp32 PSUM
accumulation), and a partial output projection against its 256-column slice
of Wo. The host sums the 8 partial outputs.

Layout notes:
 - x is pre-transposed on host to XT [D, B*S] so the contraction dim lands on
   SBUF partitions with contiguous 512B DMA runs.
 - Wq/Wk rows are permuted per head to [even features, odd features] so RoPE
   becomes a contiguous block rotation (dot products are invariant to a fixed
   permutation applied to both q and k).
 - Scores are computed transposed [t_k, t_q] so the AV matmul needs no
   transposes; softmax denominators come from a ones-vector matmul on the
   tensor engine and are folded in after AV via a K=1 broadcast matmul.
 - Softmax skips max-subtraction: scores ~ N(0,1) here, exp is safe in fp32.
"""

import math
import sys

import numpy as np

try:
    import concourse.bass as bass
except ImportError:  # pragma: no cover
    sys.path.insert(0, "/opt/trn_rl_repo")
    import concourse.bass as bass

import ml_dtypes
import concourse.mybir as mybir
import concourse.tile as tile
from concourse import bacc
from concourse.bass_utils import run_bass_kernel_spmd
from concourse.masks import make_identity

F32 = mybir.dt.float32
F32R = mybir.dt.float32r
BF16 = mybir.dt.bfloat16

B, S, D = 2, 2048, 2048
H, DH = 16, 128
N_CORES = 8
HPC = H // N_CORES  # 2 heads per core
T = B * S  # 4096
TT = S // 128  # 16 token tiles per batch
SCALE = 1.0 / math.sqrt(DH)


def _round_tf32(a):
    """Round fp32 array to fp32r-compatible mantissa (13 explicit bits)."""
    u = np.ascontiguousarray(a, dtype=np.float32).view(np.uint32)
    u = (u + np.uint32(0x1000)) & np.uint32(0xFFFFE000)
    return u.view(np.float32)


def build_nc(reps=1):
    """Build the per-core Bass program. reps>1 wraps the compute in a HW loop
    (identical work each iteration) for slope-based timing."""
    nc = bacc.Bacc("TRN2", target_bir_lowering=False, debug=False,
                   num_devices=N_CORES)

    XT = nc.dram_tensor("XT", [D, T], F32R, kind="ExternalInput")
    WALL = nc.dram_tensor("WALL", [D, 768], F32R, kind="ExternalInput")
    W2 = nc.dram_tensor("W2", [2 * DH, D], BF16, kind="ExternalInput")
    C4 = nc.dram_tensor("C4", [S, 256], F32, kind="ExternalInput")
    S4 = nc.dram_tensor("S4", [S, 256], F32, kind="ExternalInput")
    LM = nc.dram_tensor("LM", [128, 128], BF16, kind="ExternalInput")
    ONESC = nc.dram_tensor("ONESC", [128, 1], BF16, kind="ExternalInput")
    ONESR = nc.dram_tensor("ONESR", [1, 128], F32R, kind="ExternalInput")
    Y = nc.dram_tensor("Y", [T, D], F32, kind="ExternalOutput")

    XT_r = XT.rearrange("(dk p) t -> p dk t", p=128)

    with nc.allow_low_precision(reason="fp32r/bf16 matmul inputs"), \
         tile.TileContext(nc) as tc:
        with tc.tile_pool(name="res", bufs=1) as res, \
             tc.tile_pool(name="work", bufs=2) as work, \
             tc.tile_pool(name="psA", bufs=2, space="PSUM") as psA, \
             tc.tile_pool(name="psB", bufs=2, space="PSUM") as psB, \
             tc.tile_pool(name="psL", bufs=1, space="PSUM") as psL:

            # resident tensors
            wall_sb = res.tile([128, 16, 768], F32R)
            nc.sync.dma_start(wall_sb[:], WALL.rearrange("(dk p) f -> p dk f", p=128))
            w2_sb = res.tile([128, 2, D], BF16)
            nc.sync.dma_start(w2_sb[:], W2.rearrange("(h p) e -> p h e", p=128))
            c4_sb = res.tile([128, TT, 256], F32)
            nc.sync.dma_start(c4_sb[:], C4.rearrange("(tt p) j -> p tt j", p=128))
            s4_sb = res.tile([128, TT, 256], F32)
            nc.sync.dma_start(s4_sb[:], S4.rearrange("(tt p) j -> p tt j", p=128))
            lm_sb = res.tile([128, 128], BF16)
            nc.sync.dma_start(lm_sb[:], LM[:])
            onesc_sb = res.tile([128, 1], BF16)
            nc.sync.dma_start(onesc_sb[:], ONESC[:])
            onesr_sb = res.tile([1, 128], F32R)
            nc.sync.dma_start(onesr_sb[:], ONESR[:])
            ident = res.tile([128, 128], F32)
            make_identity(nc, ident[:])

            # per-batch persistent tiles (rotate via bufs=1 tags)
            def batch_tiles():
                qT = work.tile([128, HPC, S], BF16, tag="qT", bufs=1)
                kT = work.tile([128, HPC, S], BF16, tag="kT", bufs=1)
                v_sb = work.tile([128, TT, 256], BF16, tag="v", bufs=1)
                outT = work.tile([128, HPC, S], BF16, tag="outT", bufs=1)
                return qT, kT, v_sb, outT

            def stage_a(b, qT, kT, v_sb):
                for tt in range(TT):
                    gt = b * S + tt * 128
                    xcol = work.tile([128, 16, 128], F32R, tag="xcol")
                    nc.sync.dma_start(xcol[:], XT_r[:, :, gt:gt + 128])
                    ps_qk = psA.tile([128, 512], F32, tag="big")
                    ps_v = psA.tile([128, 256], F32, tag="aux")
                    for dk in range(16):
                        nc.tensor.matmul(ps_qk[:], xcol[:, dk, :],
                                         wall_sb[:, dk, 0:512],
                                         start=(dk == 0), stop=(dk == 15))
                        nc.tensor.matmul(ps_v[:], xcol[:, dk, :],
                                         wall_sb[:, dk, 512:768],
                                         start=(dk == 0), stop=(dk == 15))
                    # v: straight copy to [t,f] layout (bf16)
                    nc.vector.tensor_copy(v_sb[:, tt, :], ps_v[:])
                    # RoPE on the q/k block: blocks g in {qh0,qh1,kh0,kh1},
                    # each [top(64) | bot(64)]
                    topv = ps_qk.rearrange("p (g two j) -> p g two j", two=2, j=64)[:, :, 0, :]
                    botv = ps_qk.rearrange("p (g two j) -> p g two j", two=2, j=64)[:, :, 1, :]
                    ct = c4_sb[:, tt, :].rearrange("p (g j) -> p g j", j=64)
                    st = s4_sb[:, tt, :].rearrange("p (g j) -> p g j", j=64)
                    m1 = work.tile([128, 4, 64], F32, tag="m1")
                    m2 = work.tile([128, 4, 64], F32, tag="m2")
                    rot = work.tile([128, 512], F32, tag="rot")
                    rotv = rot.rearrange("p (g two j) -> p g two j", two=2, j=64)
                    nc.vector.tensor_mul(m1[:], topv, ct)
                    nc.vector.tensor_mul(m2[:], botv, st)
                    nc.vector.tensor_sub(rotv[:, :, 0, :], m1[:], m2[:])
                    nc.vector.tensor_mul(m1[:], botv, ct)
                    nc.vector.tensor_mul(m2[:], topv, st)
                    nc.vector.tensor_add(rotv[:, :, 1, :], m1[:], m2[:])
                    # transpose the 4 per-head blocks to [f, t] (bf16 out)
                    for g in range(4):
                        ps_t = psB.tile([128, 128], F32, tag="tr")
                        nc.tensor.transpose(ps_t[:], rot[:, g * 128:(g + 1) * 128],
                                            ident[:])
                        dst = (qT if g < 2 else kT)
                        nc.vector.tensor_copy(
                            dst[:, g % 2, tt * 128:(tt + 1) * 128], ps_t[:])

            def stage_b(b, h, qT, kT, v_sb, outT):
                for qc in range(4):
                    nkt = 4 * (qc + 1)
                    ps_o = psB.tile([128, 512], F32, tag="o", bufs=1)
                    ps_l = psL.tile([1, 512], F32, tag="l")
                    for kt in range(nkt):
                        off = max(0, (kt - 4 * qc) * 128)
                        n = 512 - off
                        ps_s = psA.tile([128, 512], F32, tag="big")
                        nc.tensor.matmul(
                            ps_s[:, off:512],
                            kT[:, h, kt * 128:(kt + 1) * 128],
                            qT[:, h, qc * 512 + off:(qc + 1) * 512],
                            start=True, stop=True)
                        p_sb = work.tile([128, 512], BF16, tag="p", bufs=3)
                        nc.scalar.activation(p_sb[:, off:512], ps_s[:, off:512],
                                             mybir.ActivationFunctionType.Exp,
                                             scale=SCALE)
                        if kt >= 4 * qc:
                            nc.vector.tensor_mul(p_sb[:, off:off + 128],
                                                 p_sb[:, off:off + 128], lm_sb[:])
                        nc.tensor.matmul(ps_l[0:1, off:512], onesc_sb[:],
                                         p_sb[:, off:512],
                                         start=(kt == 0), stop=(kt == nkt - 1))
                        nc.tensor.matmul(ps_o[:, off:512],
                                         v_sb[:, kt, h * 128:(h + 1) * 128],
                                         p_sb[:, off:512],
                                         start=(kt == 0), stop=(kt == nkt - 1))
                    recip = work.tile([1, 512], F32R, tag="rc")
                    nc.vector.reciprocal(recip[:], ps_l[0:1, :])
                    ps_bc = psA.tile([128, 512], F32, tag="aux")
                    nc.tensor.matmul(ps_bc[:], onesr_sb[:], recip[:],
                                     start=True, stop=True)
                    bc_sb = work.tile([128, 512], F32, tag="bcs")
                    nc.scalar.copy(bc_sb[:], ps_bc[:])
                    nc.vector.tensor_mul(outT[:, h, qc * 512:(qc + 1) * 512],
                                         ps_o[:], bc_sb[:])

            def stage_c(b, outT):
                for tt in range(TT):
                    gt = b * S + tt * 128
                    for ec in range(4):
                        ps_y = psA.tile([128, 512], F32, tag="big")
                        nc.tensor.matmul(ps_y[:],
                                         outT[:, 0, tt * 128:(tt + 1) * 128],
                                         w2_sb[:, 0, ec * 512:(ec + 1) * 512],
                                         start=True, stop=False)
                        nc.tensor.matmul(ps_y[:],
                                         outT[:, 1, tt * 128:(tt + 1) * 128],
                                         w2_sb[:, 1, ec * 512:(ec + 1) * 512],
                                         start=False, stop=True)
                        y_sb = work.tile([128, 512], F32, tag="ysb")
                        nc.scalar.copy(y_sb[:], ps_y[:])
                        nc.sync.dma_start(
                            Y[gt:gt + 128, ec * 512:(ec + 1) * 512], y_sb[:])

            def body():
                for b in range(B):
                    qT, kT, v_sb, outT = batch_tiles()
                    stage_a(b, qT, kT, v_sb)
                    for h in range(HPC):
                        stage_b(b, h, qT, kT, v_sb, outT)
                    stage_c(b, outT)

            if reps == 1:
                body()
            else:
                with tc.For_i(0, reps, 1):
                    body()

    nc.compile()
    return nc


def make_inputs(x, Wq, Wk, Wv, Wo):
    """Host-side sharding/prep. Returns per-core input dicts."""
    x2 = np.ascontiguousarray(x.reshape(T, D))
    xt = _round_tf32(np.ascontiguousarray(x2.T))

    inv_freq = 1.0 / (10000.0 ** (np.arange(0, DH, 2, dtype=np.float64) / DH))
    freqs = np.arange(S, dtype=np.float64)[:, None] * inv_freq[None, :]
    emb = np.concatenate([freqs, freqs], axis=1)
    cosT = np.cos(emb)[:, ::2].astype(np.float32)  # [S, 64]
    sinT = np.sin(emb)[:, ::2].astype(np.float32)
    c4 = np.ascontiguousarray(np.tile(cosT, (1, 4)))
    s4 = np.ascontiguousarray(np.tile(sinT, (1, 4)))
    lmask = (np.arange(128)[None, :] >= np.arange(128)[:, None]).astype(
        ml_dtypes.bfloat16)
    onesc = np.ones((128, 1), ml_dtypes.bfloat16)
    onesr = np.ones((1, 128), np.float32)

    in_maps = []
    for c in range(N_CORES):
        pr = []
        for h in (2 * c, 2 * c + 1):
            base = h * DH
            pr += [base + 2 * j for j in range(64)]
            pr += [base + 2 * j + 1 for j in range(64)]
        vr = list(range(2 * c * DH, 2 * c * DH + 2 * DH))
        wall = np.concatenate([Wq[pr].T, Wk[pr].T, Wv[vr].T], axis=1)
        wall = _round_tf32(wall)
        w2 = np.ascontiguousarray(Wo[:, vr].T).astype(ml_dtypes.bfloat16)
        in_maps.append({
            "XT": xt, "WALL": wall, "W2": w2, "C4": c4, "S4": s4,
            "LM": lmask, "ONESC": onesc, "ONESR": onesr,
        })
    return in_maps


_NC_CACHE = {}


def kernel(x, Wq, Wk, Wv, Wo):
    x = np.asarray(x, dtype=np.float32)
    Wq = np.asarray(Wq, dtype=np.float32)
    Wk = np.asarray(Wk, dtype=np.float32)
    Wv = np.asarray(Wv, dtype=np.float32)
    Wo = np.asarray(Wo, dtype=np.float32)

    if 1 not in _NC_CACHE:
        _NC_CACHE[1] = build_nc(1)
    nc = _NC_CACHE[1]
    in_maps = make_inputs(x, Wq, Wk, Wv, Wo)
    res = run_bass_kernel_spmd(nc, in_maps, core_ids=list(range(N_CORES)))
    y = np.zeros((T, D), np.float64)
    for c in range(N_CORES):
        y += res.results[c]["Y"].astype(np.float64)
    return y.astype(np.float32).reshape(B, S, D)


# revision 20
# speedup vs baseline: 1.6695x; 1.6695x over previous
"""Trainium2 Bass kernel for nn_MultiHeadAttention (B=2, S=2048, D=2048, H=16, Dh=128).

Sharding: tensor-parallel over heads — 2 heads per core on 8 cores.
Each core computes q/k/v projections for its 2 heads (full 2048-deep
contraction in fp32r), RoPE, causal attention (bf16 internals, fp32 PSUM
accumulation), and a partial output projection against its 256-column slice
of Wo. The host sums the 8 partial outputs.

Layout notes:
 - x is pre-transposed on host to XT [D, B*S] so the contraction dim lands on
   SBUF partitions with contiguous 512B DMA runs.
 - Wq/Wk rows are permuted per head to [even features, odd features] so RoPE
   becomes a contiguous block rotation (dot products are invariant to a fixed
   permutation applied to both q and k).
 - Scores are computed transposed [t_k, t_q] so the AV matmul needs no
   transposes; softmax denominators come from a ones-vector matmul on the
   tensor engine and are folded in after AV via a K=1 broadcast matmul.
 - Softmax skips max-subtraction: scores ~ N(0,1) here, exp is safe in fp32.
"""

import math
import sys

import numpy as np

try:
    import concourse.bass as bass
except ImportError:  # pragma: no cover
    sys.path.insert(0, "/opt/trn_rl_repo")
    import concourse.bass as bass

import ml_dtypes
import concourse.mybir as mybir
import concourse.tile as tile
from concourse import bacc
from concourse.bass_utils import run_bass_kernel_spmd
from concourse.masks import make_identity

F32 = mybir.dt.float32
F32R = mybir.dt.float32r
BF16 = mybir.dt.bfloat16
F16 = mybir.dt.float16

B, S, D = 2, 2048, 2048
H, DH = 16, 128
N_CORES = 8
HPC = H // N_CORES  # 2 heads per core
T = B * S  # 4096
TT = S // 128  # 16 token tiles per batch
SCALE = 1.0 / math.sqrt(DH)


def _round_tf32(a):
    """Round fp32 array to fp32r-compatible mantissa (13 explicit bits)."""
    u = np.ascontiguousarray(a, dtype=np.float32).view(np.uint32)
    u = (u + np.uint32(0x1000)) & np.uint32(0xFFFFE000)
    return u.view(np.float32)


def build_nc(reps=1, stages="abc", colsum=True):
    """Build the per-core Bass program. reps>1 wraps the compute in a HW loop
    (identical work each iteration) for slope-based timing."""
    nc = bacc.Bacc("TRN2", target_bir_lowering=False, debug=False,
                   num_devices=N_CORES)

    XT = nc.dram_tensor("XT", [T // 256, 128, 16, 256], BF16, kind="ExternalInput")
    WALL = nc.dram_tensor("WALL", [D, 768], BF16, kind="ExternalInput")
    W2 = nc.dram_tensor("W2", [2 * DH, D], BF16, kind="ExternalInput")
    C4 = nc.dram_tensor("C4", [S, 256], F32, kind="ExternalInput")
    S4 = nc.dram_tensor("S4", [S, 256], F32, kind="ExternalInput")
    LM = nc.dram_tensor("LM", [128, 128], BF16, kind="ExternalInput")
    ONESC = nc.dram_tensor("ONESC", [128, 1], BF16, kind="ExternalInput")
    ONESR = nc.dram_tensor("ONESR", [1, 128], F32R, kind="ExternalInput")
    Y = nc.dram_tensor("Y", [T, D], F16, kind="ExternalOutput")


    with nc.allow_low_precision(reason="bf16/fp32r matmul inputs"), \
         tile.TileContext(nc) as tc:
        with tc.tile_pool(name="res", bufs=1) as res, \
             tc.tile_pool(name="work", bufs=2) as work, \
             tc.tile_pool(name="psA", bufs=2, space="PSUM") as psA, \
             tc.tile_pool(name="psB", bufs=4, space="PSUM") as psB, \
             tc.tile_pool(name="psL", bufs=2, space="PSUM") as psL:

            # resident tensors
            wall_sb = res.tile([128, 16, 768], BF16)
            WALL_r = WALL.rearrange("(dk p) f -> p dk f", p=128)
            c4_sb = res.tile([128, TT, 256], F32)
            C4_r = C4.rearrange("(tt p) j -> p tt j", p=128)
            s4_sb = res.tile([128, TT, 256], F32)
            S4_r = S4.rearrange("(tt p) j -> p tt j", p=128)
            # interleave: wall columns arrive ahead of the rope-table chunks
            # they gate, so tile 0's matmuls and RoPE start ASAP
            for dk in range(16):
                nc.scalar.dma_start(wall_sb[:, dk, :], WALL_r[:, dk, :])
                if dk % 4 == 3:
                    ch = dk // 4
                    nc.scalar.dma_start(c4_sb[:, ch * 4:(ch + 1) * 4, :],
                                        C4_r[:, ch * 4:(ch + 1) * 4, :])
                    nc.scalar.dma_start(s4_sb[:, ch * 4:(ch + 1) * 4, :],
                                        S4_r[:, ch * 4:(ch + 1) * 4, :])
            lm_sb = res.tile([128, 128], BF16)
            nc.scalar.dma_start(lm_sb[:], LM[:])
            onesc_sb = res.tile([128, 1], BF16)
            nc.scalar.dma_start(onesc_sb[:], ONESC[:])
            onesr_sb = res.tile([1, 128], F32R)
            nc.scalar.dma_start(onesr_sb[:], ONESR[:])
            ident = res.tile([128, 128], BF16)
            make_identity(nc, ident[:])
            w2_sb = res.tile([128, 2, D], BF16)
            w2_loaded = [False]

            def load_w2():
                if not w2_loaded[0]:
                    nc.scalar.dma_start(
                        w2_sb[:], W2.rearrange("(h p) e -> p h e", p=128))
                    w2_loaded[0] = True

            def batch_tiles(b):
                qT = work.tile([128, HPC, S], BF16, tag=f"qT{b}", bufs=1,
                               name=f"qT{b}")
                kT = work.tile([128, HPC, S], BF16, tag=f"kT{b}", bufs=1,
                               name=f"kT{b}")
                v_sb = work.tile([128, TT, 256], BF16, tag=f"v{b}", bufs=1,
                                 name=f"v{b}")
                outT = work.tile([128, HPC, S], BF16, tag=f"outT{b}", bufs=1,
                                 name=f"outT{b}")
                return qT, kT, v_sb, outT

            def stage_a(b, qT, kT, v_sb):
                xcols = {}
                for t2 in range(TT // 2):
                    xc = work.tile([128, 16, 256], BF16, tag="xcol", bufs=3,
                                   name=f"xcol{t2}")
                    nc.sync.dma_start(xc[:], XT[b * (TT // 2) + t2])
                    xcols[t2] = xc
                for tt in range(TT):
                    gt = b * S + tt * 128
                    xcol = xcols[tt // 2][:, :, (tt % 2) * 128:(tt % 2) * 128 + 128]
                    ps_qk = psA.tile([128, 512], F32, tag="big")
                    ps_v = psL.tile([128, 256], F32, tag="l", bufs=2)
                    for dk in range(16):
                        nc.tensor.matmul(ps_qk[:], xcol[:, dk, :],
                                         wall_sb[:, dk, 0:512],
                                         start=(dk == 0), stop=(dk == 15))
                        nc.tensor.matmul(ps_v[:], xcol[:, dk, :],
                                         wall_sb[:, dk, 512:768],
                                         start=(dk == 0), stop=(dk == 15))
                    nc.scalar.copy(v_sb[:, tt, :], ps_v[:])
                    # RoPE: blocks g in {qh0,qh1,kh0,kh1}, each [top64 | bot64]
                    qkv = ps_qk.rearrange("p (g two j) -> p g two j", two=2, j=64)
                    topv, botv = qkv[:, :, 0, :], qkv[:, :, 1, :]
                    ct = c4_sb[:, tt, :].rearrange("p (g j) -> p g j", j=64)
                    st = s4_sb[:, tt, :].rearrange("p (g j) -> p g j", j=64)
                    m1 = work.tile([128, 4, 64], F32, tag="m1")
                    m2 = work.tile([128, 4, 64], F32, tag="m2")
                    rot = work.tile([128, 512], BF16, tag="rot")
                    rotv = rot.rearrange("p (g two j) -> p g two j", two=2, j=64)
                    nc.vector.tensor_mul(m1[:], topv, ct)
                    nc.vector.tensor_mul(m2[:], botv, st)
                    nc.vector.tensor_sub(rotv[:, :, 0, :], m1[:], m2[:])
                    nc.vector.tensor_mul(m1[:], botv, ct)
                    nc.vector.tensor_mul(m2[:], topv, st)
                    nc.vector.tensor_add(rotv[:, :, 1, :], m1[:], m2[:])
                    for g in range(4):
                        ps_t = psB.tile([128, 128], BF16, tag="trO", bufs=4)
                        nc.tensor.transpose(ps_t[:], rot[:, g * 128:(g + 1) * 128],
                                            ident[:])
                        dst = (qT if g < 2 else kT)
                        nc.scalar.copy(
                            dst[:, g % 2, tt * 128:(tt + 1) * 128], ps_t[:])

            def stage_b(streams, interleave_c=False):
                # streams: list of (qT, kT, v_sb, outT, h) quadruples
                def epilogue(outT, h, qc, ps_o, ps_l):
                    recip = work.tile([1, 512], F32R, tag="rc")
                    nc.vector.reciprocal(recip[:], ps_l[0:1, :])
                    ps_bc = psA.tile([128, 512], F32, tag="big", bufs=2)
                    nc.tensor.matmul(ps_bc[:], onesr_sb[:], recip[:],
                                     start=True, stop=True)
                    bc_sb = work.tile([128, 512], F32, tag="bcs")
                    nc.scalar.copy(bc_sb[:], ps_bc[:])
                    nc.vector.tensor_mul(outT[:, h, qc * 512:(qc + 1) * 512],
                                         ps_o[:], bc_sb[:])

                pend_c = []
                def flush_c():
                    for (bi, o, pqc) in pend_c:
                        stage_c(bi, o, pqc)
                    pend_c.clear()
                for qc in range(4):
                    nkt = 4 * (qc + 1)
                    ps_o, ps_l, acc = {}, {}, {}
                    for si in range(len(streams)):
                        ps_o[si] = psB.tile([128, 512], F32, tag="trO", bufs=4,
                                            name=f"ps_o{si}")
                        acc[si] = work.tile([128, 512], BF16, tag="acc", bufs=5,
                                            name=f"acc{si}")
                    pend_av = []  # (kt, off, si, p_sb) awaiting the AV matmul
                    def flush_av(nkt=nkt):
                        for (fkt, foff, fsi, fp) in pend_av:
                            _, _, f_v, _, fh = streams[fsi]
                            nc.tensor.matmul(
                                ps_o[fsi][:, foff:512],
                                f_v[:, fkt, fh * 128:(fh + 1) * 128],
                                fp[:, foff:512],
                                start=(fkt == 0), stop=(fkt == nkt - 1))
                        pend_av.clear()
                    for kt in range(nkt):
                        off = max(0, (kt - 4 * qc) * 128)
                        new_av = []
                        for si, (qT, kT, v_sb, outT, h) in enumerate(streams):
                            ps_s = psA.tile([128, 512], F32, tag="big", bufs=2)
                            nc.tensor.matmul(
                                ps_s[:, off:512],
                                kT[:, h, kt * 128:(kt + 1) * 128],
                                qT[:, h, qc * 512 + off:(qc + 1) * 512],
                                start=True, stop=True)
                            p_sb = work.tile([128, 512], BF16, tag="p", bufs=10)
                            nc.scalar.activation(p_sb[:, off:512], ps_s[:, off:512],
                                                 mybir.ActivationFunctionType.Exp,
                                                 scale=SCALE)
                            if kt >= 4 * qc:
                                nc.vector.tensor_mul(p_sb[:, off:off + 128],
                                                     p_sb[:, off:off + 128],
                                                     lm_sb[:])
                            if kt == 0:
                                nc.vector.tensor_copy(acc[si][:], p_sb[:])
                            else:
                                nc.vector.tensor_add(acc[si][:, off:512],
                                                     acc[si][:, off:512],
                                                     p_sb[:, off:512])
                            new_av.append((kt, off, si, p_sb))
                        flush_av()
                        pend_av.extend(new_av)
                        if kt == 0:
                            flush_c()
                    flush_av()
                    for si, (qT, kT, v_sb, outT, h) in enumerate(streams):
                        ps_l[si] = psL.tile([1, 512], F32, tag="l", bufs=2,
                                            name=f"ps_l{si}")
                        nc.tensor.matmul(ps_l[si][0:1, :], onesc_sb[:],
                                         acc[si][:], start=True, stop=True)
                    for si, (qT, kT, v_sb, outT, h) in enumerate(streams):
                        epilogue(outT, h, qc, ps_o[si], ps_l[si])
                    if interleave_c:
                        seen = []
                        for (qT, kT, v_sb, outT, h) in streams:
                            if any(o is outT for o in seen):
                                continue
                            seen.append(outT)
                        for bi, o in enumerate(seen):
                            pend_c.append((bi, o, qc))
                if interleave_c:
                    flush_c()

            def stage_c(b, outT, qc=None):
                tts = range(TT) if qc is None else range(qc * 4, qc * 4 + 4)
                for tt in tts:
                    gt = b * S + tt * 128
                    y_sb = work.tile([128, D], F16, tag="ysb")
                    for ec in range(4):
                        if ec % 2 == 0:
                            ps_y = psA.tile([128, 512], F32, tag="big", bufs=2)
                        else:
                            ps_y = psB.tile([128, 512], F32, tag="trO", bufs=4)
                        nc.tensor.matmul(ps_y[:],
                                         outT[:, 0, tt * 128:(tt + 1) * 128],
                                         w2_sb[:, 0, ec * 512:(ec + 1) * 512],
                                         start=True, stop=False)
                        nc.tensor.matmul(ps_y[:],
                                         outT[:, 1, tt * 128:(tt + 1) * 128],
                                         w2_sb[:, 1, ec * 512:(ec + 1) * 512],
                                         start=False, stop=True)
                        if ec % 2 == 0:
                            nc.scalar.copy(y_sb[:, ec * 512:(ec + 1) * 512], ps_y[:])
                        else:
                            nc.vector.tensor_copy(
                                y_sb[:, ec * 512:(ec + 1) * 512], ps_y[:])
                    nc.sync.dma_start(Y[gt:gt + 128, :], y_sb[:])

            def body():
                tiles = {}
                for b in range(B):
                    tiles[b] = batch_tiles(b)
                    stage_a(b, tiles[b][0], tiles[b][1], tiles[b][2])
                    load_w2()
                if "b" in stages:
                    streams = [(tiles[b][0], tiles[b][1], tiles[b][2],
                                tiles[b][3], h)
                               for b in range(B) for h in range(HPC)]
                    stage_b(streams, interleave_c=("c" in stages))

            if reps == 1:
                body()
            else:
                with tc.For_i(0, reps, 1):
                    body()

    nc.compile()
    return nc


def make_inputs(x, Wq, Wk, Wv, Wo):
    """Host-side sharding/prep. Returns per-core input dicts."""
    x2 = np.ascontiguousarray(x.reshape(T, D))
    xt = np.ascontiguousarray(x2.T).astype(ml_dtypes.bfloat16)
    # tile to [T/256, 128, 16, 256]: xtt[t2, p, dk, tl] = xT[dk*128+p, t2*256+tl]
    xt = np.ascontiguousarray(
        xt.reshape(16, 128, T // 256, 256).transpose(2, 1, 0, 3))

    inv_freq = 1.0 / (10000.0 ** (np.arange(0, DH, 2, dtype=np.float64) / DH))
    freqs = np.arange(S, dtype=np.float64)[:, None] * inv_freq[None, :]
    emb = np.concatenate([freqs, freqs], axis=1)
    cosT = np.cos(emb)[:, ::2].astype(np.float32)  # [S, 64]
    sinT = np.sin(emb)[:, ::2].astype(np.float32)
    c4 = np.ascontiguousarray(np.tile(cosT, (1, 4)))
    s4 = np.ascontiguousarray(np.tile(sinT, (1, 4)))
    lmask = (np.arange(128)[None, :] >= np.arange(128)[:, None]).astype(
        ml_dtypes.bfloat16)
    onesc = np.ones((128, 1), ml_dtypes.bfloat16)
    onesr = np.ones((1, 128), np.float32)

    in_maps = []
    for c in range(N_CORES):
        pr = []
        for h in (2 * c, 2 * c + 1):
            base = h * DH
            pr += [base + 2 * j for j in range(64)]
            pr += [base + 2 * j + 1 for j in range(64)]
        vr = list(range(2 * c * DH, 2 * c * DH + 2 * DH))
        wall = np.concatenate([Wq[pr].T, Wk[pr].T, Wv[vr].T],
                              axis=1).astype(ml_dtypes.bfloat16)
        w2 = np.ascontiguousarray(Wo[:, vr].T).astype(ml_dtypes.bfloat16)
        in_maps.append({
            "XT": xt, "WALL": wall, "W2": w2, "C4": c4, "S4": s4,
            "LM": lmask, "ONESC": onesc, "ONESR": onesr,
        })
    return in_maps


_NC_CACHE = {}


def kernel(x, Wq, Wk, Wv, Wo):
    x = np.asarray(x, dtype=np.float32)
    Wq = np.asarray(Wq, dtype=np.float32)
    Wk = np.asarray(Wk, dtype=np.float32)
    Wv = np.asarray(Wv, dtype=np.float32)
    Wo = np.asarray(Wo, dtype=np.float32)

    if 1 not in _NC_CACHE:
        _NC_CACHE[1] = build_nc(1)
    nc = _NC_CACHE[1]
    in_maps = make_inputs(x, Wq, Wk, Wv, Wo)
    import time as _time
    res = None
    for attempt in range(3):
        try:
            res = run_bass_kernel_spmd(nc, in_maps, core_ids=list(range(N_CORES)))
            break
        except Exception:
            # transient device wedge (NRT_EXEC_UNIT_UNRECOVERABLE) — retry
            if attempt == 2:
                raise
            _time.sleep(15)
    y = np.zeros((T, D), np.float64)
    for c in range(N_CORES):
        y += res.results[c]["Y"].astype(np.float64)
    return y.astype(np.float32).reshape(B, S, D)


# revision 22
# speedup vs baseline: 1.7043x; 1.0208x over previous
"""Trainium2 Bass kernel for nn_MultiHeadAttention (B=2, S=2048, D=2048, H=16, Dh=128).

Sharding: tensor-parallel over heads — 2 heads per core on 8 cores.
Each core computes q/k/v projections for its 2 heads (full 2048-deep
contraction in fp32r), RoPE, causal attention (bf16 internals, fp32 PSUM
accumulation), and a partial output projection against its 256-column slice
of Wo. The host sums the 8 partial outputs.

Layout notes:
 - x is pre-transposed on host to XT [D, B*S] so the contraction dim lands on
   SBUF partitions with contiguous 512B DMA runs.
 - Wq/Wk rows are permuted per head to [even features, odd features] so RoPE
   becomes a contiguous block rotation (dot products are invariant to a fixed
   permutation applied to both q and k).
 - Scores are computed transposed [t_k, t_q] so the AV matmul needs no
   transposes; softmax denominators come from a ones-vector matmul on the
   tensor engine and are folded in after AV via a K=1 broadcast matmul.
 - Softmax skips max-subtraction: scores ~ N(0,1) here, exp is safe in fp32.
"""

import math
import sys

import numpy as np

try:
    import concourse.bass as bass
except ImportError:  # pragma: no cover
    sys.path.insert(0, "/opt/trn_rl_repo")
    import concourse.bass as bass

import ml_dtypes
import concourse.mybir as mybir
import concourse.tile as tile
from concourse import bacc
from concourse.bass_utils import run_bass_kernel_spmd
from concourse.masks import make_identity

F32 = mybir.dt.float32
F32R = mybir.dt.float32r
BF16 = mybir.dt.bfloat16
F16 = mybir.dt.float16

B, S, D = 2, 2048, 2048
H, DH = 16, 128
N_CORES = 8
HPC = H // N_CORES  # 2 heads per core
T = B * S  # 4096
TT = S // 128  # 16 token tiles per batch
SCALE = 1.0 / math.sqrt(DH)


def _round_tf32(a):
    """Round fp32 array to fp32r-compatible mantissa (13 explicit bits)."""
    u = np.ascontiguousarray(a, dtype=np.float32).view(np.uint32)
    u = (u + np.uint32(0x1000)) & np.uint32(0xFFFFE000)
    return u.view(np.float32)


def build_nc(reps=1, stages="abc", colsum=True):
    """Build the per-core Bass program. reps>1 wraps the compute in a HW loop
    (identical work each iteration) for slope-based timing."""
    nc = bacc.Bacc("TRN2", target_bir_lowering=False, debug=False,
                   num_devices=N_CORES)

    XT = nc.dram_tensor("XT", [T // 256, 128, 16, 256], BF16, kind="ExternalInput")
    WALL = nc.dram_tensor("WALL", [D, 768], BF16, kind="ExternalInput")
    W2 = nc.dram_tensor("W2", [2 * DH, D], BF16, kind="ExternalInput")
    C4 = nc.dram_tensor("C4", [S, 256], F32, kind="ExternalInput")
    S4 = nc.dram_tensor("S4", [S, 256], F32, kind="ExternalInput")
    LM = nc.dram_tensor("LM", [128, 128], BF16, kind="ExternalInput")
    ONESC = nc.dram_tensor("ONESC", [128, 1], BF16, kind="ExternalInput")
    ONESR = nc.dram_tensor("ONESR", [1, 128], F32R, kind="ExternalInput")
    Y = nc.dram_tensor("Y", [T, D], F16, kind="ExternalOutput")


    with nc.allow_low_precision(reason="bf16/fp32r matmul inputs"), \
         tile.TileContext(nc) as tc:
        with tc.tile_pool(name="res", bufs=1) as res, \
             tc.tile_pool(name="work", bufs=2) as work, \
             tc.tile_pool(name="psA", bufs=2, space="PSUM") as psA, \
             tc.tile_pool(name="psB", bufs=4, space="PSUM") as psB, \
             tc.tile_pool(name="psL", bufs=2, space="PSUM") as psL:

            # resident tensors
            wall_sb = res.tile([128, 16, 768], BF16)
            WALL_r = WALL.rearrange("(dk p) f -> p dk f", p=128)
            c4_sb = res.tile([128, TT, 256], F32)
            C4_r = C4.rearrange("(tt p) j -> p tt j", p=128)
            s4_sb = res.tile([128, TT, 256], F32)
            S4_r = S4.rearrange("(tt p) j -> p tt j", p=128)
            # interleave: wall columns arrive ahead of the rope-table chunks
            # they gate, so tile 0's matmuls and RoPE start ASAP
            for dk in range(16):
                nc.scalar.dma_start(wall_sb[:, dk, :], WALL_r[:, dk, :])
                if dk % 4 == 3:
                    ch = dk // 4
                    nc.scalar.dma_start(c4_sb[:, ch * 4:(ch + 1) * 4, :],
                                        C4_r[:, ch * 4:(ch + 1) * 4, :])
                    nc.scalar.dma_start(s4_sb[:, ch * 4:(ch + 1) * 4, :],
                                        S4_r[:, ch * 4:(ch + 1) * 4, :])
            lm_sb = res.tile([128, 128], BF16)
            nc.scalar.dma_start(lm_sb[:], LM[:])
            onesc_sb = res.tile([128, 1], BF16)
            nc.scalar.dma_start(onesc_sb[:], ONESC[:])
            onesr_sb = res.tile([1, 128], F32R)
            nc.scalar.dma_start(onesr_sb[:], ONESR[:])
            ident = res.tile([128, 128], BF16)
            make_identity(nc, ident[:])
            w2_sb = res.tile([128, 2, D], BF16)
            w2_loaded = [False]

            def load_w2():
                if not w2_loaded[0]:
                    nc.scalar.dma_start(
                        w2_sb[:], W2.rearrange("(h p) e -> p h e", p=128))
                    w2_loaded[0] = True

            def batch_tiles(b):
                qT = work.tile([128, HPC, S], BF16, tag=f"qT{b}", bufs=1,
                               name=f"qT{b}")
                kT = work.tile([128, HPC, S], BF16, tag=f"kT{b}", bufs=1,
                               name=f"kT{b}")
                v_sb = work.tile([128, TT, 256], BF16, tag=f"v{b}", bufs=1,
                                 name=f"v{b}")
                outT = work.tile([128, HPC, S], BF16, tag=f"outT{b}", bufs=1,
                                 name=f"outT{b}")
                return qT, kT, v_sb, outT

            def stage_a(b, qT, kT, v_sb):
                xcols = {}
                for t2 in range(TT // 2):
                    xc = work.tile([128, 16, 256], BF16, tag="xcol", bufs=3,
                                   name=f"xcol{t2}")
                    nc.sync.dma_start(xc[:], XT[b * (TT // 2) + t2])
                    xcols[t2] = xc
                for tt in range(TT):
                    gt = b * S + tt * 128
                    xcol = xcols[tt // 2][:, :, (tt % 2) * 128:(tt % 2) * 128 + 128]
                    ps_qk = psA.tile([128, 512], F32, tag="big")
                    ps_v = psL.tile([128, 256], F32, tag="l", bufs=2)
                    for dk in range(16):
                        nc.tensor.matmul(ps_qk[:], xcol[:, dk, :],
                                         wall_sb[:, dk, 0:512],
                                         start=(dk == 0), stop=(dk == 15))
                        nc.tensor.matmul(ps_v[:], xcol[:, dk, :],
                                         wall_sb[:, dk, 512:768],
                                         start=(dk == 0), stop=(dk == 15))
                    nc.scalar.copy(v_sb[:, tt, :], ps_v[:])
                    # RoPE: blocks g in {qh0,qh1,kh0,kh1}, each [top64 | bot64]
                    qkv = ps_qk.rearrange("p (g two j) -> p g two j", two=2, j=64)
                    topv, botv = qkv[:, :, 0, :], qkv[:, :, 1, :]
                    ct = c4_sb[:, tt, :].rearrange("p (g j) -> p g j", j=64)
                    st = s4_sb[:, tt, :].rearrange("p (g j) -> p g j", j=64)
                    m1 = work.tile([128, 4, 64], F32, tag="m1")
                    m2 = work.tile([128, 4, 64], F32, tag="m2")
                    rot = work.tile([128, 512], BF16, tag="rot")
                    rotv = rot.rearrange("p (g two j) -> p g two j", two=2, j=64)
                    nc.vector.tensor_mul(m1[:], topv, ct)
                    nc.vector.tensor_mul(m2[:], botv, st)
                    nc.vector.tensor_sub(rotv[:, :, 0, :], m1[:], m2[:])
                    nc.vector.tensor_mul(m1[:], botv, ct)
                    nc.vector.tensor_mul(m2[:], topv, st)
                    nc.vector.tensor_add(rotv[:, :, 1, :], m1[:], m2[:])
                    for g in range(4):
                        ps_t = psB.tile([128, 128], BF16, tag="trO", bufs=4)
                        nc.tensor.transpose(ps_t[:], rot[:, g * 128:(g + 1) * 128],
                                            ident[:])
                        dst = (qT if g < 2 else kT)
                        nc.scalar.copy(
                            dst[:, g % 2, tt * 128:(tt + 1) * 128], ps_t[:])

            def stage_b(streams, interleave_c=False):
                # streams: list of (qT, kT, v_sb, outT, h) quadruples
                def epilogue(outT, h, qc, ps_o, ps_l):
                    recip = work.tile([1, 512], F32R, tag="rc")
                    nc.vector.reciprocal(recip[:], ps_l[0:1, :])
                    ps_bc = psA.tile([128, 512], F32, tag="big", bufs=2)
                    nc.tensor.matmul(ps_bc[:], onesr_sb[:], recip[:],
                                     start=True, stop=True)
                    bc_sb = work.tile([128, 512], F32, tag="bcs")
                    nc.scalar.copy(bc_sb[:], ps_bc[:])
                    nc.vector.tensor_mul(outT[:, h, qc * 512:(qc + 1) * 512],
                                         ps_o[:], bc_sb[:])

                pend_c = []
                def flush_c():
                    for (bi, o, pqc) in pend_c:
                        stage_c(bi, o, pqc)
                    pend_c.clear()
                for qc in range(4):
                    nkt = 4 * (qc + 1)
                    ps_o, ps_l, acc = {}, {}, {}
                    for si in range(len(streams)):
                        ps_o[si] = psB.tile([128, 512], F32, tag="trO", bufs=4,
                                            name=f"ps_o{si}")
                        acc[si] = work.tile([128, 512], BF16, tag="acc", bufs=5,
                                            name=f"acc{si}")
                    pend_av = []  # (kt, off, si, p_sb) awaiting the AV matmul
                    def flush_av(nkt=nkt):
                        for (fkt, foff, fsi, fp) in pend_av:
                            _, _, f_v, _, fh = streams[fsi]
                            nc.tensor.matmul(
                                ps_o[fsi][:, foff:512],
                                f_v[:, fkt, fh * 128:(fh + 1) * 128],
                                fp[:, foff:512],
                                start=(fkt == 0), stop=(fkt == nkt - 1))
                        pend_av.clear()
                    for kt in range(nkt):
                        off = max(0, (kt - 4 * qc) * 128)
                        new_av = []
                        for si, (qT, kT, v_sb, outT, h) in enumerate(streams):
                            if si % 2 == 0:
                                ps_s = psA.tile([128, 512], F32, tag="big",
                                                bufs=2, name=f"ps_s{si}")
                            else:
                                ps_s = psL.tile([128, 512], F32, tag="l",
                                                bufs=2, name=f"ps_s{si}")
                            nc.tensor.matmul(
                                ps_s[:, off:512],
                                kT[:, h, kt * 128:(kt + 1) * 128],
                                qT[:, h, qc * 512 + off:(qc + 1) * 512],
                                start=True, stop=True)
                            p_sb = work.tile([128, 512], BF16, tag="p", bufs=10)
                            nc.scalar.activation(p_sb[:, off:512], ps_s[:, off:512],
                                                 mybir.ActivationFunctionType.Exp,
                                                 scale=SCALE)
                            if kt >= 4 * qc:
                                nc.vector.tensor_mul(p_sb[:, off:off + 128],
                                                     p_sb[:, off:off + 128],
                                                     lm_sb[:])
                            if kt == 0:
                                nc.vector.tensor_copy(acc[si][:], p_sb[:])
                            else:
                                nc.vector.tensor_add(acc[si][:, off:512],
                                                     acc[si][:, off:512],
                                                     p_sb[:, off:512])
                            new_av.append((kt, off, si, p_sb))
                        flush_av()
                        pend_av.extend(new_av)
                        if kt == 0:
                            flush_c()
                    flush_av()
                    for si, (qT, kT, v_sb, outT, h) in enumerate(streams):
                        ps_l[si] = psL.tile([1, 512], F32, tag="l", bufs=2,
                                            name=f"ps_l{si}")
                        nc.tensor.matmul(ps_l[si][0:1, :], onesc_sb[:],
                                         acc[si][:], start=True, stop=True)
                    for si, (qT, kT, v_sb, outT, h) in enumerate(streams):
                        epilogue(outT, h, qc, ps_o[si], ps_l[si])
                    if interleave_c:
                        seen = []
                        for (qT, kT, v_sb, outT, h) in streams:
                            if any(o is outT for o in seen):
                                continue
                            seen.append(outT)
                        for bi, o in enumerate(seen):
                            pend_c.append((bi, o, qc))
                if interleave_c:
                    flush_c()

            def stage_c(b, outT, qc=None):
                tts = range(TT) if qc is None else range(qc * 4, qc * 4 + 4)
                for tt in tts:
                    gt = b * S + tt * 128
                    y_sb = work.tile([128, D], F16, tag="ysb")
                    for ec in range(4):
                        if ec % 2 == 0:
                            ps_y = psA.tile([128, 512], F32, tag="big", bufs=2)
                        else:
                            ps_y = psB.tile([128, 512], F32, tag="trO", bufs=4)
                        nc.tensor.matmul(ps_y[:],
                                         outT[:, 0, tt * 128:(tt + 1) * 128],
                                         w2_sb[:, 0, ec * 512:(ec + 1) * 512],
                                         start=True, stop=False)
                        nc.tensor.matmul(ps_y[:],
                                         outT[:, 1, tt * 128:(tt + 1) * 128],
                                         w2_sb[:, 1, ec * 512:(ec + 1) * 512],
                                         start=False, stop=True)
                        if ec % 2 == 0:
                            nc.scalar.copy(y_sb[:, ec * 512:(ec + 1) * 512], ps_y[:])
                        else:
                            nc.vector.tensor_copy(
                                y_sb[:, ec * 512:(ec + 1) * 512], ps_y[:])
                    nc.sync.dma_start(Y[gt:gt + 128, :], y_sb[:])

            def body():
                tiles = {}
                for b in range(B):
                    tiles[b] = batch_tiles(b)
                    stage_a(b, tiles[b][0], tiles[b][1], tiles[b][2])
                    load_w2()
                if "b" in stages:
                    streams = [(tiles[b][0], tiles[b][1], tiles[b][2],
                                tiles[b][3], h)
                               for b in range(B) for h in range(HPC)]
                    stage_b(streams, interleave_c=("c" in stages))

            if reps == 1:
                body()
            else:
                with tc.For_i(0, reps, 1):
                    body()

    nc.compile()
    return nc


def make_inputs(x, Wq, Wk, Wv, Wo):
    """Host-side sharding/prep. Returns per-core input dicts."""
    x2 = np.ascontiguousarray(x.reshape(T, D))
    xt = np.ascontiguousarray(x2.T).astype(ml_dtypes.bfloat16)
    # tile to [T/256, 128, 16, 256]: xtt[t2, p, dk, tl] = xT[dk*128+p, t2*256+tl]
    xt = np.ascontiguousarray(
        xt.reshape(16, 128, T // 256, 256).transpose(2, 1, 0, 3))

    inv_freq = 1.0 / (10000.0 ** (np.arange(0, DH, 2, dtype=np.float64) / DH))
    freqs = np.arange(S, dtype=np.float64)[:, None] * inv_freq[None, :]
    emb = np.concatenate([freqs, freqs], axis=1)
    cosT = np.cos(emb)[:, ::2].astype(np.float32)  # [S, 64]
    sinT = np.sin(emb)[:, ::2].astype(np.float32)
    c4 = np.ascontiguousarray(np.tile(cosT, (1, 4)))
    s4 = np.ascontiguousarray(np.tile(sinT, (1, 4)))
    lmask = (np.arange(128)[None, :] >= np.arange(128)[:, None]).astype(
        ml_dtypes.bfloat16)
    onesc = np.ones((128, 1), ml_dtypes.bfloat16)
    onesr = np.ones((1, 128), np.float32)

    in_maps = []
    for c in range(N_CORES):
        pr = []
        for h in (2 * c, 2 * c + 1):
            base = h * DH
            pr += [base + 2 * j for j in range(64)]
            pr += [base + 2 * j + 1 for j in range(64)]
        vr = list(range(2 * c * DH, 2 * c * DH + 2 * DH))
        wall = np.concatenate([Wq[pr].T, Wk[pr].T, Wv[vr].T],
                              axis=1).astype(ml_dtypes.bfloat16)
        w2 = np.ascontiguousarray(Wo[:, vr].T).astype(ml_dtypes.bfloat16)
        in_maps.append({
            "XT": xt, "WALL": wall, "W2": w2, "C4": c4, "S4": s4,
            "LM": lmask, "ONESC": onesc, "ONESR": onesr,
        })
    return in_maps


_NC_CACHE = {}


def kernel(x, Wq, Wk, Wv, Wo):
    x = np.asarray(x, dtype=np.float32)
    Wq = np.asarray(Wq, dtype=np.float32)
    Wk = np.asarray(Wk, dtype=np.float32)
    Wv = np.asarray(Wv, dtype=np.float32)
    Wo = np.asarray(Wo, dtype=np.float32)

    if 1 not in _NC_CACHE:
        _NC_CACHE[1] = build_nc(1)
    nc = _NC_CACHE[1]
    in_maps = make_inputs(x, Wq, Wk, Wv, Wo)
    import time as _time
    res = None
    for attempt in range(3):
        try:
            res = run_bass_kernel_spmd(nc, in_maps, core_ids=list(range(N_CORES)))
            break
        except Exception:
            # transient device wedge (NRT_EXEC_UNIT_UNRECOVERABLE) — retry
            if attempt == 2:
                raise
            _time.sleep(15)
    y = np.zeros((T, D), np.float64)
    for c in range(N_CORES):
        y += res.results[c]["Y"].astype(np.float64)
    return y.astype(np.float32).reshape(B, S, D)
